# revision 1
# baseline (speedup 1.0000x reference)
"""MCorr1d Trainium2 kernel (8 NeuronCores).

Problem (hardcoded from spec):
  in_    [1024, 64, 512]  fp32   (X, N, C_in)
  weight [16, 512, 512]   fp32   (KW, C_in, C_out)
  bias   [512]            fp32
  out    [64, 64, 512]    fp32   (Y, N, C_out)

  out[y, n, o] = bias[o] + sum_{w=0}^{15} sum_c in_[(y+1)*(w+1)-1, n, c] * weight[w, c, o]

Sharding: data-parallel over N (batch): core i handles n in [8*i, 8*i+8).
Each core computes rows r = y*8 + n_local (512 rows) of out[., n_slice, .]
as 16 accumulating GEMMs of [512,512] @ [512,512] plus a rank-1 bias term.

Host packs, per core, A_pack[w, c, r] = in_[(y+1)*(w+1)-1, n0+n, c]
(im2col-style gather+transpose) so every DMA is contiguous and the
tensor engine consumes tiles directly with no on-device transposes.
Both A and W stream per tap (no reuse within one execution), so compute
starts after ~2MB of DMA and everything overlaps.

Precision modes:
  fp32   : plain float32 matmuls (4 cycles/row on PE), rel err ~1e-6
  fp32r  : float32r single-pass matmuls (1 cycle/row), rel err ~1.6e-4
  bf16x3 : hi/lo bf16 split, 3 matmuls (hi@hi + hi@lo + lo@hi), rel err ~5e-6
  bf16   : plain bf16 (half DMA bytes), rel err ~2e-3
"""

import contextlib

import numpy as np

X_LEN, N_BATCH, C_IN = 1024, 64, 512
KW, C_OUT = 16, 512
Y_OUT = 64
N_CORES = 8
N_PER = N_BATCH // N_CORES  # 8
ROWS = Y_OUT * N_PER  # 512
KC = C_IN // 128  # 4 k-chunks
MC = ROWS // 128  # 4 m-chunks

MODE = "bf16x3"

_XS = np.array([[(y + 1) * (w + 1) - 1 for y in range(Y_OUT)] for w in range(KW)])


def _build_nc(mode):
    return _build_nc_reps(mode, 1)


def _build_nc_reps(mode, reps, loop_n=0):
    import concourse.mybir as mybir
    import concourse.tile as tile
    from concourse import bacc

    f32 = mybir.dt.float32
    if mode == "fp32":
        mdt = f32
    elif mode == "fp32r":
        mdt = mybir.dt.float32r
    elif mode in ("bf16x3", "bf16"):
        mdt = mybir.dt.bfloat16
    else:
        raise ValueError(mode)

    nc = bacc.Bacc("TRN2", target_bir_lowering=False, debug=False,
                   num_devices=N_CORES)

    # Per-core DRAM tensors (SPMD: same program, different data per core).
    ins = {}
    names = (("a_hi", "a_lo", "w_hi", "w_lo") if mode == "bf16x3"
             else ("a", "w"))
    for nm in names:
        shp = [KW, C_IN, ROWS] if nm.startswith("a") else [KW, C_IN, C_OUT]
        ins[nm] = nc.dram_tensor(nm, shp, mdt, kind="ExternalInput").ap()
    bias_t = nc.dram_tensor("bias", [1, C_OUT], mybir.dt.float32r,
                            kind="ExternalInput").ap()
    ones_t = nc.dram_tensor("ones", [1, 128], mybir.dt.float32r,
                            kind="ExternalInput").ap()
    out_t = nc.dram_tensor("out", [ROWS, C_OUT], f32, kind="ExternalOutput").ap()

    with tile.TileContext(nc) as tc:
        with tc.tile_pool(name="asb", bufs=4) as asb, \
             tc.tile_pool(name="csb", bufs=1) as csb, \
             tc.tile_pool(name="osb", bufs=2) as osb, \
             tc.tile_pool(name="ps", bufs=1, space="PSUM") as ps:

            # Constants
            bias_sb = csb.tile([1, C_OUT], mybir.dt.float32r, tag="bias")
            nc.sync.dma_start(bias_sb[:], bias_t[:])
            ones_sb = csb.tile([1, 128], mybir.dt.float32r, tag="ones")
            nc.sync.dma_start(ones_sb[:], ones_t[:])

            loop_cm = (tc.For_i(0, loop_n, 1) if loop_n
                       else contextlib.nullcontext())
            with loop_cm:
                for _rep in range(reps):
                    _emit_body(nc, mode, mdt, f32, asb, osb, ps,
                               bias_sb, ones_sb, ins, out_t)

    nc.compile()
    return nc


def _emit_body(nc, mode, mdt, f32, asb, osb, ps, bias_sb, ones_sb, ins, out_t):
    # Output accumulators: 4 PSUM banks of [128, 512]
    acc = [ps.tile([128, C_OUT], f32, name=f"acc{m}", tag=f"acc{m}")
           for m in range(MC)]

    # Bias as rank-1 matmul opens each accumulation group.
    for m in range(MC):
        nc.tensor.matmul(acc[m][:], ones_sb[:], bias_sb[:],
                         start=True, stop=False)

    for w in range(KW):
        tiles = {}
        for nm, ap in ins.items():
            fd = ROWS if nm.startswith("a") else C_OUT
            t = asb.tile([128, KC, fd], mdt, name=nm + "_t", tag=nm)
            nc.sync.dma_start(t[:], ap[w].rearrange("(k p) f -> p k f", p=128))
            tiles[nm] = t
        if mode == "bf16x3":
            pairs = [(tiles["a_hi"], tiles["w_hi"]),
                     (tiles["a_hi"], tiles["w_lo"]),
                     (tiles["a_lo"], tiles["w_hi"])]
        else:
            pairs = [(tiles["a"], tiles["w"])]
        last_w = (w == KW - 1)
        for m in range(MC):
            for k in range(KC):
                for pi, (at, wt) in enumerate(pairs):
                    stop = (last_w and k == KC - 1 and pi == len(pairs) - 1)
                    nc.tensor.matmul(
                        acc[m][:],
                        at[:, k, m * 128:(m + 1) * 128],
                        wt[:, k, :],
                        start=False, stop=stop)

    for m in range(MC):
        o_sb = osb.tile([128, C_OUT], f32, tag="o")
        nc.vector.tensor_copy(o_sb[:], acc[m][:])
        nc.sync.dma_start(out_t[m * 128:(m + 1) * 128, :], o_sb[:])


_NC_CACHE = {}


def _get_nc(mode):
    if mode not in _NC_CACHE:
        _NC_CACHE[mode] = _build_nc(mode)
    return _NC_CACHE[mode]


def _pack_inputs(in_, weight, bias, mode):
    """Host-side gather/transpose pack. Returns list of per-core input maps."""
    import ml_dtypes

    in_ = np.asarray(in_, dtype=np.float32)
    weight = np.asarray(weight, dtype=np.float32)
    bias = np.asarray(bias, dtype=np.float32)

    # G[w, y, n, c] = in_[(y+1)(w+1)-1, n, c]
    G = in_[_XS.reshape(-1)].reshape(KW, Y_OUT, N_BATCH, C_IN)
    # A_all[w, c, y, n]
    A_all = np.ascontiguousarray(G.transpose(0, 3, 1, 2))

    ones = np.ones((1, 128), np.float32)
    bias2 = bias.reshape(1, C_OUT)

    def split(x):
        hi = x.astype(ml_dtypes.bfloat16)
        lo = (x - hi.astype(np.float32)).astype(ml_dtypes.bfloat16)
        return hi, lo

    if mode == "bf16x3":
        w_hi, w_lo = split(weight)
    elif mode == "bf16":
        w_b = weight.astype(ml_dtypes.bfloat16)

    in_maps = []
    for c in range(N_CORES):
        n0 = c * N_PER
        a_c = np.ascontiguousarray(
            A_all[:, :, :, n0:n0 + N_PER]).reshape(KW, C_IN, ROWS)
        m = {"bias": bias2, "ones": ones}
        if mode == "bf16x3":
            a_hi, a_lo = split(a_c)
            m.update(a_hi=a_hi, a_lo=a_lo, w_hi=w_hi, w_lo=w_lo)
        elif mode == "bf16":
            m.update(a=a_c.astype(ml_dtypes.bfloat16), w=w_b)
        else:
            m.update(a=a_c, w=weight)
        in_maps.append(m)
    return in_maps


def kernel(in_, weight, bias):
    from concourse.bass_utils import run_bass_kernel_spmd

    nc = _get_nc(MODE)
    in_maps = _pack_inputs(in_, weight, bias, MODE)
    res = run_bass_kernel_spmd(nc, in_maps, core_ids=list(range(N_CORES)))
    # Each core returns out [ROWS, C_OUT] with rows = y*N_PER + n_local.
    parts = [r["out"].reshape(Y_OUT, N_PER, C_OUT) for r in res.results]
    return np.concatenate(parts, axis=1).astype(np.float32)


# revision 2
# speedup vs baseline: 24235.0709x; 24235.0709x over previous
"""MCorr1d Trainium2 kernel (8 NeuronCores).

Problem (hardcoded from spec):
  in_    [1024, 64, 512]  fp32   (X, N, C_in)
  weight [16, 512, 512]   fp32   (KW, C_in, C_out)
  bias   [512]            fp32
  out    [64, 64, 512]    fp32   (Y, N, C_out)

  out[y, n, o] = bias[o] + sum_{w=0}^{15} sum_c in_[(y+1)*(w+1)-1, n, c] * weight[w, c, o]

Sharding: data-parallel over N (batch): core i handles n in [8*i, 8*i+8).
Each core computes rows r = y*8 + n_local (512 rows) of out[., n_slice, .]
as 16 accumulating GEMMs of [512,512] @ [512,512] plus a rank-1 bias term.

Host packs, per core, A_pack[w, c, r] = in_[(y+1)*(w+1)-1, n0+n, c]
(im2col-style gather+transpose) so every DMA is contiguous and the
tensor engine consumes tiles directly with no on-device transposes.
Both A and W stream per tap (no reuse within one execution), so compute
starts after ~2MB of DMA and everything overlaps.

Precision modes:
  fp32   : plain float32 matmuls (4 cycles/row on PE), rel err ~1e-6
  fp32r  : float32r single-pass matmuls (1 cycle/row), rel err ~1.6e-4
  bf16x3 : hi/lo bf16 split, 3 matmuls (hi@hi + hi@lo + lo@hi), rel err ~5e-6
  bf16   : plain bf16 (half DMA bytes), rel err ~2e-3
"""

import contextlib

import numpy as np

X_LEN, N_BATCH, C_IN = 1024, 64, 512
KW, C_OUT = 16, 512
Y_OUT = 64
N_CORES = 8
N_PER = N_BATCH // N_CORES  # 8
ROWS = Y_OUT * N_PER  # 512
KC = C_IN // 128  # 4 k-chunks
MC = ROWS // 128  # 4 m-chunks

MODE = "bf16x3"

_XS = np.array([[(y + 1) * (w + 1) - 1 for y in range(Y_OUT)] for w in range(KW)])


def _build_nc(mode):
    return _build_nc_reps(mode, 1)


def _build_nc_reps(mode, reps, loop_n=0):
    import concourse.mybir as mybir
    import concourse.tile as tile
    from concourse import bacc

    f32 = mybir.dt.float32
    if mode == "fp32":
        mdt = f32
    elif mode == "fp32r":
        mdt = mybir.dt.float32r
    elif mode in ("bf16x3", "bf16"):
        mdt = mybir.dt.bfloat16
    else:
        raise ValueError(mode)

    nc = bacc.Bacc("TRN2", target_bir_lowering=False, debug=False,
                   num_devices=N_CORES)

    # Per-core DRAM tensors (SPMD: same program, different data per core).
    ins = {}
    names = (("a_hi", "w_hi", "w_lo", "a_lo") if mode == "bf16x3"
             else ("a", "w"))
    for nm in names:
        shp = [KW, C_IN, ROWS] if nm.startswith("a") else [KW, C_IN, C_OUT]
        ins[nm] = nc.dram_tensor(nm, shp, mdt, kind="ExternalInput").ap()
    bias_t = nc.dram_tensor("bias", [1, C_OUT], mybir.dt.float32r,
                            kind="ExternalInput").ap()
    ones_t = nc.dram_tensor("ones", [1, 128], mybir.dt.float32r,
                            kind="ExternalInput").ap()
    out_t = nc.dram_tensor("out", [ROWS, C_OUT], f32, kind="ExternalOutput").ap()

    with tile.TileContext(nc) as tc:
        with tc.tile_pool(name="asb", bufs=4) as asb, \
             tc.tile_pool(name="csb", bufs=1) as csb, \
             tc.tile_pool(name="osb", bufs=2) as osb, \
             tc.tile_pool(name="ps", bufs=1, space="PSUM") as ps:

            # Constants
            bias_sb = csb.tile([1, C_OUT], mybir.dt.float32r, tag="bias")
            nc.sync.dma_start(bias_sb[:], bias_t[:])
            ones_sb = csb.tile([1, 128], mybir.dt.float32r, tag="ones")
            nc.sync.dma_start(ones_sb[:], ones_t[:])

            loop_cm = (tc.For_i(0, loop_n, 1) if loop_n
                       else contextlib.nullcontext())
            with loop_cm:
                for _rep in range(reps):
                    _emit_body(nc, mode, mdt, f32, asb, osb, ps,
                               bias_sb, ones_sb, ins, out_t)

    nc.compile()
    return nc


def _emit_body(nc, mode, mdt, f32, asb, osb, ps, bias_sb, ones_sb, ins, out_t):
    # Output accumulators: 4 PSUM banks of [128, 512]
    acc = [ps.tile([128, C_OUT], f32, name=f"acc{m}", tag=f"acc{m}")
           for m in range(MC)]

    # Bias as rank-1 matmul opens each accumulation group.
    for m in range(MC):
        nc.tensor.matmul(acc[m][:], ones_sb[:], bias_sb[:],
                         start=True, stop=False)

    for w in range(KW):
        tiles = {}
        for nm, ap in ins.items():
            fd = ROWS if nm.startswith("a") else C_OUT
            t = asb.tile([128, KC, fd], mdt, name=nm + "_t", tag=nm)
            nc.sync.dma_start(t[:], ap[w].rearrange("(k p) f -> p k f", p=128))
            tiles[nm] = t
        if mode == "bf16x3":
            pairs = [(tiles["a_hi"], tiles["w_hi"]),
                     (tiles["a_hi"], tiles["w_lo"]),
                     (tiles["a_lo"], tiles["w_hi"])]
        else:
            pairs = [(tiles["a"], tiles["w"])]
        last_w = (w == KW - 1)
        for m in range(MC):
            for k in range(KC):
                for pi, (at, wt) in enumerate(pairs):
                    stop = (last_w and k == KC - 1 and pi == len(pairs) - 1)
                    nc.tensor.matmul(
                        acc[m][:],
                        at[:, k, m * 128:(m + 1) * 128],
                        wt[:, k, :],
                        start=False, stop=stop)

    for m in range(MC):
        o_sb = osb.tile([128, C_OUT], f32, tag="o")
        nc.vector.tensor_copy(o_sb[:], acc[m][:])
        nc.sync.dma_start(out_t[m * 128:(m + 1) * 128, :], o_sb[:])


_NC_CACHE = {}


def _get_nc(mode):
    if mode not in _NC_CACHE:
        _NC_CACHE[mode] = _build_nc(mode)
    return _NC_CACHE[mode]


def _pack_inputs(in_, weight, bias, mode):
    """Host-side gather/transpose pack. Returns list of per-core input maps."""
    import ml_dtypes

    in_ = np.asarray(in_, dtype=np.float32)
    weight = np.asarray(weight, dtype=np.float32)
    bias = np.asarray(bias, dtype=np.float32)

    # G[w, y, n, c] = in_[(y+1)(w+1)-1, n, c]
    G = in_[_XS.reshape(-1)].reshape(KW, Y_OUT, N_BATCH, C_IN)
    # A_all[w, c, y, n]
    A_all = np.ascontiguousarray(G.transpose(0, 3, 1, 2))

    ones = np.ones((1, 128), np.float32)
    bias2 = bias.reshape(1, C_OUT)

    def split(x):
        hi = x.astype(ml_dtypes.bfloat16)
        lo = (x - hi.astype(np.float32)).astype(ml_dtypes.bfloat16)
        return hi, lo

    if mode == "bf16x3":
        w_hi, w_lo = split(weight)
    elif mode == "bf16":
        w_b = weight.astype(ml_dtypes.bfloat16)

    in_maps = []
    for c in range(N_CORES):
        n0 = c * N_PER
        a_c = np.ascontiguousarray(
            A_all[:, :, :, n0:n0 + N_PER]).reshape(KW, C_IN, ROWS)
        m = {"bias": bias2, "ones": ones}
        if mode == "bf16x3":
            a_hi, a_lo = split(a_c)
            m.update(a_hi=a_hi, a_lo=a_lo, w_hi=w_hi, w_lo=w_lo)
        elif mode == "bf16":
            m.update(a=a_c.astype(ml_dtypes.bfloat16), w=w_b)
        else:
            m.update(a=a_c, w=weight)
        in_maps.append(m)
    return in_maps


def kernel(in_, weight, bias):
    from concourse.bass_utils import run_bass_kernel_spmd

    nc = _get_nc(MODE)
    in_maps = _pack_inputs(in_, weight, bias, MODE)
    res = run_bass_kernel_spmd(nc, in_maps, core_ids=list(range(N_CORES)))
    # Each core returns out [ROWS, C_OUT] with rows = y*N_PER + n_local.
    parts = [r["out"].reshape(Y_OUT, N_PER, C_OUT) for r in res.results]
    return np.concatenate(parts, axis=1).astype(np.float32)
